# revision 13
# baseline (speedup 1.0000x reference)
"""Trainium2 Bass kernel for nn_FCVI_Net_78864189489850.

Computation (reference):
  L = lower-tri scatter of cov_vector (exp on diag)          [769, 769]
  samples = mean + L @ z                                      [769, S, B]
  W0 = samples[0:256], b0 = samples[256:512],
  W1 = samples[512:768], b1 = samples[768]
  h = relu(x * W0 + b0);  out = sum_o h * W1 + b1             [S, B]

Strategy (8 NeuronCores, batch-sharded, no cross-device comms):
  - Host builds L, transposes to LT, casts to f16.  Each core gets a
    B-shard of z (columns c = s*256 + b_local, 4096 cols) in f16, PLUS
    256 extra rows x*z[0:256] ("z0") so the PE accumulates
    x*sT0 + sT1 straight into PSUM.
  - delta-trick: host solves LT[:, 512:768]^T delta = mean2 (min-norm)
    and ships z+delta.  The W1-side matmul then lands sT2 + mean2 in
    PSUM directly; the spurious delta terms in the W0/b0 region are
    batch-independent constants folded into apar on the host.
  - Single PSUM bank per c-tile, psAB[128, 512] = [x*sT0+sT1 | sT2+m2];
    adjacent LT column ranges stream in ONE matmul, 8 matmuls per
    c-tile, 2688 streamed PE columns (exact triangular trim).
  - Epilogue: DVE u2 = psAB[0:256] + apar; ACT h = relu(u2);
    DVE STT accumulates stag[:, m] = sum_o h * psAB[256:512].
  - The b1 row (mean768 + L[768,:] @ z) is a host-side bias added in
    _assemble; it is 0.13% of the FLOPs.
  - Output staged [128, 32], DMA'd out in 4 chunks; host reassembles
    [16, 2048] and adds b1.
"""
import os
import numpy as np

P = 769
S = 16
B = 2048
NCORES = 8
BC = B // NCORES          # 256 batch per core
NCOL = S * BC             # 4096 columns per core
NCT = NCOL // 128         # 32 c-tiles per core
ZR = 1024                 # za rows: 768 z + 256 x-scaled z

# LT columns kept per k-tile t (LT[k, i] == 0 for i < k; col 768 on host)
LT_COLS = [(0, 768), (128, 768), (256, 768), (384, 768),
           (512, 768), (640, 768)]
LT_OFF = [0]
for _lo, _hi in LT_COLS:
    LT_OFF.append(LT_OFF[-1] + (_hi - _lo))
LTW = LT_OFF[-1]          # 2688 packed LT columns
# z column chunks: small head so compute starts early, wide steady-state
CHUNKS = [(0, 128), (128, 128), (256, 256), (512, 512),
          (1024, 1024), (2048, 2048)]

_cache = {}


def _mm_dtype():
    import concourse.mybir as mybir
    name = os.environ.get("BASS_FCVI_DTYPE", "f16")
    return {
        "f16": (mybir.dt.float16, np.float16),
        "f32r": (mybir.dt.float32r, np.float32),
    }[name]


def _build_program():
    import concourse.bacc as bacc
    import concourse.tile as tile
    from concourse import mybir

    mmdt, _ = _mm_dtype()
    f32 = mybir.dt.float32

    nc = bacc.Bacc("TRN2", target_bir_lowering=False, debug=False)

    za_d = nc.dram_tensor("za", [ZR, NCOL], mmdt, kind="ExternalInput")
    lt_d = nc.dram_tensor("lt", [128, LTW], mmdt, kind="ExternalInput")
    cst_d = nc.dram_tensor("cst", [128, 512], f32, kind="ExternalInput")
    out_d = nc.dram_tensor("out", [128, NCT], f32, kind="ExternalOutput")

    with tile.TileContext(nc) as tc:
        with (
            tc.tile_pool(name="zpool", bufs=1) as zpool,
            tc.tile_pool(name="ltpool", bufs=1) as ltpool,
            tc.tile_pool(name="cpool", bufs=1) as cpool,
            tc.tile_pool(name="work", bufs=4) as work,
            tc.tile_pool(name="gsc", bufs=3) as gsc,
            tc.tile_pool(name="ps", bufs=6, space="PSUM") as ps_pool,
        ):
            zc = [None] * len(CHUNKS)
            zc0h = [None, None]

            def load_zc(q, eng):
                cs, cn = CHUNKS[q]
                zq = zpool.tile([128, 8, cn], mmdt, tag=f"zc{q}")
                src = za_d.ap()[:, cs:cs + cn].rearrange(
                    "(t p) c -> p t c", p=128)
                eng.dma_start(out=zq[:], in_=src)
                zc[q] = zq

            # lt is host-packed to [128, 2688]: one descriptor.  Chunk 0 of z
            # splits into k-slot halves so the first matmul's operand (slot 0)
            # lands before the z0 slots.  Parallel issue: lt+cst on the
            # scalar queue, z on sync.
            lt = ltpool.tile([128, LTW], mmdt, tag="lt")
            nc.scalar.dma_start(out=lt[:], in_=lt_d.ap()[:, :])
            for h_ in range(2):
                zq = zpool.tile([128, 4, 128], mmdt, tag=f"zc0{h_}")
                src = za_d.ap()[h_ * 512:(h_ + 1) * 512, 0:128].rearrange(
                    "(t p) c -> p t c", p=128)
                nc.sync.dma_start(out=zq[:], in_=src)
                zc0h[h_] = zq

            cst = cpool.tile([128, 512], f32, tag="cst")
            nc.scalar.dma_start(out=cst[:], in_=cst_d.ap()[:, :])
            apar = [cst[:, 0:256], cst[:, 256:512]]

            for q in range(1, len(CHUNKS)):
                load_zc(q, nc.sync)

            stag = cpool.tile([128, NCT], f32, tag="stag")

            def rhs(t, g0, g1):
                lo, _ = LT_COLS[t]
                return lt[:, LT_OFF[t] + g0 - lo:LT_OFF[t] + g1 - lo]

            qi = 0
            for m in range(NCT):
                par = m % 2
                if m * 128 >= CHUNKS[qi][0] + CHUNKS[qi][1]:
                    qi += 1
                cl = m * 128 - CHUNKS[qi][0]

                def lhsT(t):
                    if qi == 0:
                        return zc0h[t // 4][:, t % 4, 0:128]
                    return zc[qi][:, t, cl:cl + 128]

                # psAB[:, o]     = x*sT0[o] + sT1[o] + delta consts
                # psAB[:, 256+j] = sT2[j] + mean2[j]
                # LT cols [256, 768) of k-tiles 0..5 map linearly onto
                # psAB cols [0, 512): one stream covers both halves.
                pq = ps_pool.tile([128, 512], f32, tag="ps")
                nc.tensor.matmul(pq[:, 0:512], lhsT(0), rhs(0, 256, 768),
                                 start=True, stop=False)
                nc.tensor.matmul(pq[:, 0:256], lhsT(6), rhs(0, 0, 256),
                                 start=False, stop=False)
                nc.tensor.matmul(pq[:, 0:512], lhsT(1), rhs(1, 256, 768),
                                 start=False, stop=False)
                nc.tensor.matmul(pq[:, 128:256], lhsT(7),
                                 lt[:, LT_OFF[1]:LT_OFF[1] + 128],
                                 start=False, stop=False)
                nc.tensor.matmul(pq[:, 0:512], lhsT(2), rhs(2, 256, 768),
                                 start=False, stop=False)
                nc.tensor.matmul(pq[:, 128:512], lhsT(3), rhs(3, 384, 768),
                                 start=False, stop=False)
                nc.tensor.matmul(pq[:, 256:512], lhsT(4), rhs(4, 512, 768),
                                 start=False, stop=False)
                nc.tensor.matmul(pq[:, 384:512], lhsT(5), rhs(5, 640, 768),
                                 start=False, stop=True)

                # u2 = x*W0 + b0 (+mean terms)   (DVE)
                u2 = work.tile([128, 256], f32, tag="u2")
                nc.vector.tensor_add(u2[:], pq[:, 0:256], apar[par])
                # h = relu(u2)   (ACT)
                h = work.tile([128, 256], f32, tag="h")
                nc.scalar.activation(h[:], u2[:],
                                     mybir.ActivationFunctionType.Relu)
                # stag[:, m] = sum_o h * (sT2 + mean2)   (DVE STT accumulate)
                gA = gsc.tile([128, 256], f32, tag="gA")
                nc.vector.scalar_tensor_tensor(
                    out=gA[:], in0=pq[:, 256:512], scalar=1.0, in1=h[:],
                    op0=mybir.AluOpType.mult, op1=mybir.AluOpType.mult,
                    accum_out=stag[:, m:m + 1])

                if m % 8 == 7:
                    sl = slice(m - 7, m + 1)
                    nc.sync.dma_start(out=out_d.ap()[:, sl], in_=stag[:, sl])

    nc.compile()
    return nc


def _prep_inputs(x, mean, cov_vector, z):
    _, npdt = _mm_dtype()

    L = np.zeros((P, P), dtype=np.float32)
    L[np.tril_indices(P)] = cov_vector
    d = np.diag(L).copy()
    L[np.diag_indices(P)] = np.exp(d)

    ltf = L.T[:768, :768].astype(np.float64)              # LT[k, i] = L[i, k]
    # delta-trick: min-norm solve of LT[:, 512:768]^T delta = mean2 so the
    # shifted z lands sT2 + mean2 in PSUM.
    m2 = mean[512:768].astype(np.float64)
    delta, *_ = np.linalg.lstsq(ltf[:, 512:768].T, m2, rcond=None)
    w = delta @ ltf                                        # [768] spurious sums
    assert np.abs(w[512:768] - m2).max() < 1e-6 * max(1.0, np.abs(m2).max())
    delta32 = delta.astype(np.float32)

    # lt packed [128, 2688]: k-tile t's stripe LT[128t:128t+128, lo:hi]
    ltT = L.T.astype(npdt)
    lt = np.empty((128, LT_OFF[-1]), dtype=npdt)
    for t, (lo, hi) in enumerate(LT_COLS):
        lt[:, LT_OFF[t]:LT_OFF[t + 1]] = ltT[t * 128:(t + 1) * 128, lo:hi]
    lt = np.ascontiguousarray(lt)

    z2 = z.reshape(P, S, B).astype(np.float32, copy=False)
    # b1 bias row on host: b1[s, b] = mean[768] + sum_k L[768, k] z[k, s, b]
    b1 = mean[768] + np.tensordot(L[768, :], z2, axes=1)  # [S, B]

    w = w.astype(np.float32)
    in_maps = []
    for c in range(NCORES):
        zs = z2[:, :, c * BC:(c + 1) * BC].reshape(P, NCOL)
        xs = x[c * BC:(c + 1) * BC].astype(np.float32)
        zd = zs[:768] + delta32[:, None]
        za = np.empty((ZR, NCOL), dtype=npdt)
        za[:768] = zd.astype(npdt)
        # x-prescaled z rows: column c = s*BC + b pairs with x[b]
        xcol = np.tile(xs, S)                             # [NCOL]
        za[768:1024] = (zd[:256] * xcol[None, :]).astype(npdt)
        cst = np.empty((128, 512), dtype=np.float32)
        # apar[p, o] = x_p*(mean0_o - w_o) + (mean1_o - w_{256+o}),
        # one block per batch parity
        a0 = mean[0:256] - w[0:256]
        a1 = mean[256:512] - w[256:512]
        cst[:, 0:256] = xs[0:128, None] * a0[None, :] + a1[None, :]
        cst[:, 256:512] = xs[128:256, None] * a0[None, :] + a1[None, :]
        in_maps.append({"za": za, "lt": lt, "cst": cst})
    return in_maps, b1


def _assemble(results, b1):
    out = np.empty((S, B), dtype=np.float32)
    for c in range(NCORES):
        o = results[c]["out"]                       # [128, 32]
        oc = o.reshape(128, S, 2).transpose(1, 2, 0).reshape(S, BC)
        out[:, c * BC:(c + 1) * BC] = oc
    out += b1
    return out


def _run(inputs, trace=False, trace_kwargs=None):
    from concourse.bass_utils import run_bass_kernel_spmd

    key = os.environ.get("BASS_FCVI_DTYPE", "f16")
    if key not in _cache:
        _cache[key] = _build_program()
    nc = _cache[key]

    in_maps, b1 = _prep_inputs(**inputs)
    kw = {}
    if trace:
        kw["trace"] = True
        if trace_kwargs:
            kw.update(trace_kwargs)
    res = run_bass_kernel_spmd(nc, in_maps, core_ids=list(range(NCORES)), **kw)
    return _assemble(res.results, b1), res


def kernel(x, mean, cov_vector, z):
    out, _ = _run(dict(x=np.asarray(x), mean=np.asarray(mean),
                       cov_vector=np.asarray(cov_vector), z=np.asarray(z)))
    return out


# revision 16
# speedup vs baseline: 1.0281x; 1.0281x over previous
"""Trainium2 Bass kernel for nn_FCVI_Net_78864189489850.

Computation (reference):
  L = lower-tri scatter of cov_vector (exp on diag)          [769, 769]
  samples = mean + L @ z                                      [769, S, B]
  W0 = samples[0:256], b0 = samples[256:512],
  W1 = samples[512:768], b1 = samples[768]
  h = relu(x * W0 + b0);  out = sum_o h * W1 + b1             [S, B]

Strategy (8 NeuronCores, batch-sharded, no cross-device comms):
  - Host builds L, transposes to LT, casts to f16.  Each core gets a
    B-shard of z (columns c = s*256 + b_local, 4096 cols) in f16, PLUS
    256 extra rows x*z[0:256] ("z0") so the PE accumulates
    x*sT0 + sT1 straight into PSUM.
  - delta-trick: host solves LT[:, 512:768]^T delta = mean2 (min-norm)
    and ships z+delta.  The W1-side matmul then lands sT2 + mean2 in
    PSUM directly; the spurious delta terms in the W0/b0 region are
    batch-independent constants folded into apar on the host.
  - Single PSUM bank per c-tile, psAB[128, 512] = [x*sT0+sT1 | sT2+m2];
    adjacent LT column ranges stream in ONE matmul, 8 matmuls per
    c-tile, 2688 streamed PE columns (exact triangular trim).
  - Epilogue: DVE u2 = psAB[0:256] + apar; ACT h = relu(u2);
    DVE STT accumulates stag[:, m] = sum_o h * psAB[256:512].
  - The b1 row (mean768 + L[768,:] @ z) is a host-side bias added in
    _assemble; it is 0.13% of the FLOPs.
  - Output staged [128, 32], DMA'd out in 4 chunks; host reassembles
    [16, 2048] and adds b1.
"""
import os
import numpy as np

P = 769
S = 16
B = 2048
NCORES = 8
BC = B // NCORES          # 256 batch per core
NCOL = S * BC             # 4096 columns per core
NCT = NCOL // 128         # 32 c-tiles per core
ZR = 1024                 # za rows: 768 z + 256 x-scaled z

# LT columns kept per k-tile t (LT[k, i] == 0 for i < k; col 768 on host)
LT_COLS = [(0, 768), (128, 768), (256, 768), (384, 768),
           (512, 768), (640, 768)]
LT_OFF = [0]
for _lo, _hi in LT_COLS:
    LT_OFF.append(LT_OFF[-1] + (_hi - _lo))
LTW = LT_OFF[-1]          # 2688 packed LT columns
# z column chunks: small head so compute starts early, wide steady-state
CHUNKS = [(0, 128), (128, 128), (256, 256), (512, 512),
          (1024, 1024), (2048, 1024), (3072, 1024)]

_cache = {}


def _mm_dtype():
    import concourse.mybir as mybir
    name = os.environ.get("BASS_FCVI_DTYPE", "f16")
    return {
        "f16": (mybir.dt.float16, np.float16),
        "f32r": (mybir.dt.float32r, np.float32),
    }[name]


def _build_program():
    import concourse.bacc as bacc
    import concourse.tile as tile
    from concourse import mybir

    mmdt, _ = _mm_dtype()
    f32 = mybir.dt.float32

    nc = bacc.Bacc("TRN2", target_bir_lowering=False, debug=False)

    za_d = nc.dram_tensor("za", [ZR, NCOL], mmdt, kind="ExternalInput")
    lt_d = nc.dram_tensor("lt", [128, LTW], mmdt, kind="ExternalInput")
    cst_d = nc.dram_tensor("cst", [128, 512], f32, kind="ExternalInput")
    out_d = nc.dram_tensor("out", [128, NCT], f32, kind="ExternalOutput")

    with tile.TileContext(nc) as tc:
        with (
            tc.tile_pool(name="zpool", bufs=1) as zpool,
            tc.tile_pool(name="ltpool", bufs=1) as ltpool,
            tc.tile_pool(name="cpool", bufs=1) as cpool,
            tc.tile_pool(name="work", bufs=4) as work,
            tc.tile_pool(name="gsc", bufs=3) as gsc,
            tc.tile_pool(name="ps", bufs=6, space="PSUM") as ps_pool,
        ):
            zc = [None] * len(CHUNKS)

            def load_zc(q, eng):
                cs, cn = CHUNKS[q]
                zq = zpool.tile([128, 8, cn], mmdt, tag=f"zc{q}")
                src = za_d.ap()[:, cs:cs + cn].rearrange(
                    "(t p) c -> p t c", p=128)
                eng.dma_start(out=zq[:], in_=src)
                zc[q] = zq

            # lt is host-packed to [128, 2688]; load it as one slab tile via
            # per-k-tile column slices so the first matmul's slice (k-tile 0)
            # lands before the rest.  Parallel issue: lt+cst on the scalar
            # queue, z on sync.
            lt = ltpool.tile([128, LTW], mmdt, tag="lt")
            nc.scalar.dma_start(out=lt[:, 0:LT_OFF[1]],
                                in_=lt_d.ap()[:, 0:LT_OFF[1]])
            load_zc(0, nc.sync)
            for t in range(1, 6):
                nc.scalar.dma_start(out=lt[:, LT_OFF[t]:LT_OFF[t + 1]],
                                    in_=lt_d.ap()[:, LT_OFF[t]:LT_OFF[t + 1]])

            cst = cpool.tile([128, 512], f32, tag="cst")
            nc.scalar.dma_start(out=cst[:], in_=cst_d.ap()[:, :])
            apar = [cst[:, 0:256], cst[:, 256:512]]

            for q in range(1, len(CHUNKS)):
                load_zc(q, nc.sync)

            stag = cpool.tile([128, NCT], f32, tag="stag")

            def rhs(t, g0, g1):
                lo, _ = LT_COLS[t]
                return lt[:, LT_OFF[t] + g0 - lo:LT_OFF[t] + g1 - lo]

            qi = 0
            for m in range(NCT):
                par = m % 2
                if m * 128 >= CHUNKS[qi][0] + CHUNKS[qi][1]:
                    qi += 1
                cl = m * 128 - CHUNKS[qi][0]

                def lhsT(t):
                    return zc[qi][:, t, cl:cl + 128]

                # psAB[:, o]     = x*sT0[o] + sT1[o] + delta consts
                # psAB[:, 256+j] = sT2[j] + mean2[j]
                # LT cols [256, 768) of k-tiles 0..5 map linearly onto
                # psAB cols [0, 512): one stream covers both halves.
                pq = ps_pool.tile([128, 512], f32, tag="ps")
                nc.tensor.matmul(pq[:, 0:512], lhsT(0), rhs(0, 256, 768),
                                 start=True, stop=False)
                nc.tensor.matmul(pq[:, 0:256], lhsT(6), rhs(0, 0, 256),
                                 start=False, stop=False)
                nc.tensor.matmul(pq[:, 0:512], lhsT(1), rhs(1, 256, 768),
                                 start=False, stop=False)
                nc.tensor.matmul(pq[:, 128:256], lhsT(7),
                                 lt[:, LT_OFF[1]:LT_OFF[1] + 128],
                                 start=False, stop=False)
                nc.tensor.matmul(pq[:, 0:512], lhsT(2), rhs(2, 256, 768),
                                 start=False, stop=False)
                nc.tensor.matmul(pq[:, 128:512], lhsT(3), rhs(3, 384, 768),
                                 start=False, stop=False)
                nc.tensor.matmul(pq[:, 256:512], lhsT(4), rhs(4, 512, 768),
                                 start=False, stop=False)
                nc.tensor.matmul(pq[:, 384:512], lhsT(5), rhs(5, 640, 768),
                                 start=False, stop=True)

                # u2 = x*W0 + b0 (+mean terms)   (DVE)
                u2 = work.tile([128, 256], f32, tag="u2")
                nc.vector.tensor_add(u2[:], pq[:, 0:256], apar[par])
                # h = relu(u2)   (ACT)
                h = work.tile([128, 256], f32, tag="h")
                nc.scalar.activation(h[:], u2[:],
                                     mybir.ActivationFunctionType.Relu)
                # stag[:, m] = sum_o h * (sT2 + mean2)   (DVE STT accumulate)
                gA = gsc.tile([128, 256], f32, tag="gA")
                nc.vector.scalar_tensor_tensor(
                    out=gA[:], in0=pq[:, 256:512], scalar=1.0, in1=h[:],
                    op0=mybir.AluOpType.mult, op1=mybir.AluOpType.mult,
                    accum_out=stag[:, m:m + 1])

                if m % 8 == 7:
                    sl = slice(m - 7, m + 1)
                    nc.sync.dma_start(out=out_d.ap()[:, sl], in_=stag[:, sl])

    nc.compile()
    return nc


def _prep_inputs(x, mean, cov_vector, z):
    _, npdt = _mm_dtype()

    L = np.zeros((P, P), dtype=np.float32)
    L[np.tril_indices(P)] = cov_vector
    d = np.diag(L).copy()
    L[np.diag_indices(P)] = np.exp(d)

    ltf = L.T[:768, :768].astype(np.float64)              # LT[k, i] = L[i, k]
    # delta-trick: min-norm solve of LT[:, 512:768]^T delta = mean2 so the
    # shifted z lands sT2 + mean2 in PSUM.
    m2 = mean[512:768].astype(np.float64)
    delta, *_ = np.linalg.lstsq(ltf[:, 512:768].T, m2, rcond=None)
    w = delta @ ltf                                        # [768] spurious sums
    assert np.abs(w[512:768] - m2).max() < 1e-6 * max(1.0, np.abs(m2).max())
    delta32 = delta.astype(np.float32)

    # lt packed [128, 2688]: k-tile t's stripe LT[128t:128t+128, lo:hi]
    ltT = L.T.astype(npdt)
    lt = np.empty((128, LT_OFF[-1]), dtype=npdt)
    for t, (lo, hi) in enumerate(LT_COLS):
        lt[:, LT_OFF[t]:LT_OFF[t + 1]] = ltT[t * 128:(t + 1) * 128, lo:hi]
    lt = np.ascontiguousarray(lt)

    z2 = z.reshape(P, S, B).astype(np.float32, copy=False)
    # b1 bias row on host: b1[s, b] = mean[768] + sum_k L[768, k] z[k, s, b]
    b1 = mean[768] + np.tensordot(L[768, :], z2, axes=1)  # [S, B]

    w = w.astype(np.float32)
    in_maps = []
    for c in range(NCORES):
        zs = z2[:, :, c * BC:(c + 1) * BC].reshape(P, NCOL)
        xs = x[c * BC:(c + 1) * BC].astype(np.float32)
        zd = zs[:768] + delta32[:, None]
        za = np.empty((ZR, NCOL), dtype=npdt)
        za[:768] = zd.astype(npdt)
        # x-prescaled z rows: column c = s*BC + b pairs with x[b]
        xcol = np.tile(xs, S)                             # [NCOL]
        za[768:1024] = (zd[:256] * xcol[None, :]).astype(npdt)
        cst = np.empty((128, 512), dtype=np.float32)
        # apar[p, o] = x_p*(mean0_o - w_o) + (mean1_o - w_{256+o}),
        # one block per batch parity
        a0 = mean[0:256] - w[0:256]
        a1 = mean[256:512] - w[256:512]
        cst[:, 0:256] = xs[0:128, None] * a0[None, :] + a1[None, :]
        cst[:, 256:512] = xs[128:256, None] * a0[None, :] + a1[None, :]
        in_maps.append({"za": za, "lt": lt, "cst": cst})
    return in_maps, b1


def _assemble(results, b1):
    out = np.empty((S, B), dtype=np.float32)
    for c in range(NCORES):
        o = results[c]["out"]                       # [128, 32]
        oc = o.reshape(128, S, 2).transpose(1, 2, 0).reshape(S, BC)
        out[:, c * BC:(c + 1) * BC] = oc
    out += b1
    return out


def _run(inputs, trace=False, trace_kwargs=None):
    from concourse.bass_utils import run_bass_kernel_spmd

    key = os.environ.get("BASS_FCVI_DTYPE", "f16")
    if key not in _cache:
        _cache[key] = _build_program()
    nc = _cache[key]

    in_maps, b1 = _prep_inputs(**inputs)
    kw = {}
    if trace:
        kw["trace"] = True
        if trace_kwargs:
            kw.update(trace_kwargs)
    res = run_bass_kernel_spmd(nc, in_maps, core_ids=list(range(NCORES)), **kw)
    return _assemble(res.results, b1), res


def kernel(x, mean, cov_vector, z):
    out, _ = _run(dict(x=np.asarray(x), mean=np.asarray(mean),
                       cov_vector=np.asarray(cov_vector), z=np.asarray(z)))
    return out


# revision 17
# speedup vs baseline: 1.0617x; 1.0327x over previous
"""Trainium2 Bass kernel for nn_FCVI_Net_78864189489850.

Computation (reference):
  L = lower-tri scatter of cov_vector (exp on diag)          [769, 769]
  samples = mean + L @ z                                      [769, S, B]
  W0 = samples[0:256], b0 = samples[256:512],
  W1 = samples[512:768], b1 = samples[768]
  h = relu(x * W0 + b0);  out = sum_o h * W1 + b1             [S, B]

Strategy (8 NeuronCores, batch-sharded, no cross-device comms):
  - Host builds L, transposes to LT, casts to f16.  Each core gets a
    B-shard of z (columns c = s*256 + b_local, 4096 cols) in f16, PLUS
    256 extra rows x*z[0:256] ("z0") so the PE accumulates
    x*sT0 + sT1 straight into PSUM.
  - delta-trick: host solves LT[:, 512:768]^T delta = mean2 (min-norm)
    and ships z+delta.  The W1-side matmul then lands sT2 + mean2 in
    PSUM directly; the spurious delta terms in the W0/b0 region are
    batch-independent constants folded into apar on the host.
  - Single PSUM bank per c-tile, psAB[128, 512] = [x*sT0+sT1 | sT2+m2];
    adjacent LT column ranges stream in ONE matmul, 8 matmuls per
    c-tile, 2688 streamed PE columns (exact triangular trim).
  - Epilogue: DVE u2 = psAB[0:256] + apar; ACT h = relu(u2);
    DVE STT accumulates stag[:, m] = sum_o h * psAB[256:512].
  - The b1 row (mean768 + L[768,:] @ z) is a host-side bias added in
    _assemble; it is 0.13% of the FLOPs.
  - Output staged [128, 32], DMA'd out in 4 chunks; host reassembles
    [16, 2048] and adds b1.
"""
import os
import numpy as np

P = 769
S = 16
B = 2048
NCORES = 8
BC = B // NCORES          # 256 batch per core
NCOL = S * BC             # 4096 columns per core
NCT = NCOL // 128         # 32 c-tiles per core
ZR = 1024                 # za rows: 768 z + 256 x-scaled z

# LT columns kept per k-tile t (LT[k, i] == 0 for i < k; col 768 on host)
LT_COLS = [(0, 768), (128, 768), (256, 768), (384, 768),
           (512, 768), (640, 768)]
LT_OFF = [0]
for _lo, _hi in LT_COLS:
    LT_OFF.append(LT_OFF[-1] + (_hi - _lo))
LTW = LT_OFF[-1]          # 2688 packed LT columns
# z column chunks: small head so compute starts early; DMA is nearly
# rate-matched with the PE in the first half, so keep chunks small enough
# that arrival bursts never stall the consuming c-tile.
CHUNKS = [(0, 128), (128, 128), (256, 256), (512, 256), (768, 256),
          (1024, 512), (1536, 512), (2048, 512), (2560, 512),
          (3072, 512), (3584, 512)]

_cache = {}


def _mm_dtype():
    import concourse.mybir as mybir
    name = os.environ.get("BASS_FCVI_DTYPE", "f16")
    return {
        "f16": (mybir.dt.float16, np.float16),
        "f32r": (mybir.dt.float32r, np.float32),
    }[name]


def _build_program():
    import concourse.bacc as bacc
    import concourse.tile as tile
    from concourse import mybir

    mmdt, _ = _mm_dtype()
    f32 = mybir.dt.float32

    nc = bacc.Bacc("TRN2", target_bir_lowering=False, debug=False)

    za_d = nc.dram_tensor("za", [ZR, NCOL], mmdt, kind="ExternalInput")
    lt_d = nc.dram_tensor("lt", [128, LTW], mmdt, kind="ExternalInput")
    cst_d = nc.dram_tensor("cst", [128, 512], f32, kind="ExternalInput")
    out_d = nc.dram_tensor("out", [128, NCT], f32, kind="ExternalOutput")

    with tile.TileContext(nc) as tc:
        with (
            tc.tile_pool(name="zpool", bufs=1) as zpool,
            tc.tile_pool(name="ltpool", bufs=1) as ltpool,
            tc.tile_pool(name="cpool", bufs=1) as cpool,
            tc.tile_pool(name="work", bufs=4) as work,
            tc.tile_pool(name="gsc", bufs=3) as gsc,
            tc.tile_pool(name="ps", bufs=6, space="PSUM") as ps_pool,
        ):
            zc = [None] * len(CHUNKS)

            def load_zc(q, eng):
                cs, cn = CHUNKS[q]
                zq = zpool.tile([128, 8, cn], mmdt, tag=f"zc{q}")
                src = za_d.ap()[:, cs:cs + cn].rearrange(
                    "(t p) c -> p t c", p=128)
                eng.dma_start(out=zq[:], in_=src)
                zc[q] = zq

            # lt is host-packed to [128, 2688]; load it as one slab tile via
            # per-k-tile column slices so the first matmul's slice (k-tile 0)
            # lands before the rest.  Parallel issue: lt+cst on the scalar
            # queue, z on sync.
            lt = ltpool.tile([128, LTW], mmdt, tag="lt")
            nc.scalar.dma_start(out=lt[:, 0:LT_OFF[1]],
                                in_=lt_d.ap()[:, 0:LT_OFF[1]])
            load_zc(0, nc.sync)
            for t in range(1, 6):
                nc.scalar.dma_start(out=lt[:, LT_OFF[t]:LT_OFF[t + 1]],
                                    in_=lt_d.ap()[:, LT_OFF[t]:LT_OFF[t + 1]])

            cst = cpool.tile([128, 512], f32, tag="cst")
            nc.scalar.dma_start(out=cst[:], in_=cst_d.ap()[:, :])
            apar = [cst[:, 0:256], cst[:, 256:512]]

            for q in range(1, len(CHUNKS)):
                load_zc(q, nc.sync)

            stag = cpool.tile([128, NCT], f32, tag="stag")

            def rhs(t, g0, g1):
                lo, _ = LT_COLS[t]
                return lt[:, LT_OFF[t] + g0 - lo:LT_OFF[t] + g1 - lo]

            qi = 0
            for m in range(NCT):
                par = m % 2
                if m * 128 >= CHUNKS[qi][0] + CHUNKS[qi][1]:
                    qi += 1
                cl = m * 128 - CHUNKS[qi][0]

                def lhsT(t):
                    return zc[qi][:, t, cl:cl + 128]

                # psAB[:, o]     = x*sT0[o] + sT1[o] + delta consts
                # psAB[:, 256+j] = sT2[j] + mean2[j]
                # LT cols [256, 768) of k-tiles 0..5 map linearly onto
                # psAB cols [0, 512): one stream covers both halves.
                pq = ps_pool.tile([128, 512], f32, tag="ps")
                nc.tensor.matmul(pq[:, 0:512], lhsT(0), rhs(0, 256, 768),
                                 start=True, stop=False)
                nc.tensor.matmul(pq[:, 0:256], lhsT(6), rhs(0, 0, 256),
                                 start=False, stop=False)
                nc.tensor.matmul(pq[:, 0:512], lhsT(1), rhs(1, 256, 768),
                                 start=False, stop=False)
                nc.tensor.matmul(pq[:, 128:256], lhsT(7),
                                 lt[:, LT_OFF[1]:LT_OFF[1] + 128],
                                 start=False, stop=False)
                nc.tensor.matmul(pq[:, 0:512], lhsT(2), rhs(2, 256, 768),
                                 start=False, stop=False)
                nc.tensor.matmul(pq[:, 128:512], lhsT(3), rhs(3, 384, 768),
                                 start=False, stop=False)
                nc.tensor.matmul(pq[:, 256:512], lhsT(4), rhs(4, 512, 768),
                                 start=False, stop=False)
                nc.tensor.matmul(pq[:, 384:512], lhsT(5), rhs(5, 640, 768),
                                 start=False, stop=True)

                # u2 = x*W0 + b0 (+mean terms)   (DVE)
                u2 = work.tile([128, 256], f32, tag="u2")
                nc.vector.tensor_add(u2[:], pq[:, 0:256], apar[par])
                # h = relu(u2)   (ACT)
                h = work.tile([128, 256], f32, tag="h")
                nc.scalar.activation(h[:], u2[:],
                                     mybir.ActivationFunctionType.Relu)
                # stag[:, m] = sum_o h * (sT2 + mean2)   (DVE STT accumulate)
                gA = gsc.tile([128, 256], f32, tag="gA")
                nc.vector.scalar_tensor_tensor(
                    out=gA[:], in0=pq[:, 256:512], scalar=1.0, in1=h[:],
                    op0=mybir.AluOpType.mult, op1=mybir.AluOpType.mult,
                    accum_out=stag[:, m:m + 1])

                if m % 8 == 7:
                    sl = slice(m - 7, m + 1)
                    nc.sync.dma_start(out=out_d.ap()[:, sl], in_=stag[:, sl])

    nc.compile()
    return nc


def _prep_inputs(x, mean, cov_vector, z):
    _, npdt = _mm_dtype()

    L = np.zeros((P, P), dtype=np.float32)
    L[np.tril_indices(P)] = cov_vector
    d = np.diag(L).copy()
    L[np.diag_indices(P)] = np.exp(d)

    ltf = L.T[:768, :768].astype(np.float64)              # LT[k, i] = L[i, k]
    # delta-trick: min-norm solve of LT[:, 512:768]^T delta = mean2 so the
    # shifted z lands sT2 + mean2 in PSUM.
    m2 = mean[512:768].astype(np.float64)
    delta, *_ = np.linalg.lstsq(ltf[:, 512:768].T, m2, rcond=None)
    w = delta @ ltf                                        # [768] spurious sums
    assert np.abs(w[512:768] - m2).max() < 1e-6 * max(1.0, np.abs(m2).max())
    delta32 = delta.astype(np.float32)

    # lt packed [128, 2688]: k-tile t's stripe LT[128t:128t+128, lo:hi]
    ltT = L.T.astype(npdt)
    lt = np.empty((128, LT_OFF[-1]), dtype=npdt)
    for t, (lo, hi) in enumerate(LT_COLS):
        lt[:, LT_OFF[t]:LT_OFF[t + 1]] = ltT[t * 128:(t + 1) * 128, lo:hi]
    lt = np.ascontiguousarray(lt)

    z2 = z.reshape(P, S, B).astype(np.float32, copy=False)
    # b1 bias row on host: b1[s, b] = mean[768] + sum_k L[768, k] z[k, s, b]
    b1 = mean[768] + np.tensordot(L[768, :], z2, axes=1)  # [S, B]

    w = w.astype(np.float32)
    in_maps = []
    for c in range(NCORES):
        zs = z2[:, :, c * BC:(c + 1) * BC].reshape(P, NCOL)
        xs = x[c * BC:(c + 1) * BC].astype(np.float32)
        zd = zs[:768] + delta32[:, None]
        za = np.empty((ZR, NCOL), dtype=npdt)
        za[:768] = zd.astype(npdt)
        # x-prescaled z rows: column c = s*BC + b pairs with x[b]
        xcol = np.tile(xs, S)                             # [NCOL]
        za[768:1024] = (zd[:256] * xcol[None, :]).astype(npdt)
        cst = np.empty((128, 512), dtype=np.float32)
        # apar[p, o] = x_p*(mean0_o - w_o) + (mean1_o - w_{256+o}),
        # one block per batch parity
        a0 = mean[0:256] - w[0:256]
        a1 = mean[256:512] - w[256:512]
        cst[:, 0:256] = xs[0:128, None] * a0[None, :] + a1[None, :]
        cst[:, 256:512] = xs[128:256, None] * a0[None, :] + a1[None, :]
        in_maps.append({"za": za, "lt": lt, "cst": cst})
    return in_maps, b1


def _assemble(results, b1):
    out = np.empty((S, B), dtype=np.float32)
    for c in range(NCORES):
        o = results[c]["out"]                       # [128, 32]
        oc = o.reshape(128, S, 2).transpose(1, 2, 0).reshape(S, BC)
        out[:, c * BC:(c + 1) * BC] = oc
    out += b1
    return out


def _run(inputs, trace=False, trace_kwargs=None):
    from concourse.bass_utils import run_bass_kernel_spmd

    key = os.environ.get("BASS_FCVI_DTYPE", "f16")
    if key not in _cache:
        _cache[key] = _build_program()
    nc = _cache[key]

    in_maps, b1 = _prep_inputs(**inputs)
    kw = {}
    if trace:
        kw["trace"] = True
        if trace_kwargs:
            kw.update(trace_kwargs)
    res = run_bass_kernel_spmd(nc, in_maps, core_ids=list(range(NCORES)), **kw)
    return _assemble(res.results, b1), res


def kernel(x, mean, cov_vector, z):
    out, _ = _run(dict(x=np.asarray(x), mean=np.asarray(mean),
                       cov_vector=np.asarray(cov_vector), z=np.asarray(z)))
    return out
